# revision 1
# baseline (speedup 1.0000x reference)
"""Trainium2 Bass kernel for nn_MessageGeneratorRNN.

Math (per batch row n, per step t):
    h = tanh(W_ih @ e + b_ih + W_hh @ h_prev + b_hh)
    z = W_out @ h + b_out + g_t
    x = softmax(z)                      -> output slice  [N, NOS, VOCAB]
    e = W_emb @ x + b_emb

Strategy:
  - Data-parallel over the flattened batch N = 4096: 512 rows per core, 8 cores,
    weights replicated, no collectives.
  - On-chip everything lives TRANSPOSED, [feature, batch]: weights are the
    stationary matmul operand (pre-transposed on host), activations stream, so
    no on-device transposes are ever needed.  The gumbels input and the x
    output are pre/post-transposed on the host for the same reason.
  - Softmax over the vocab (partition) axis: column sums via a ones-vector
    matmul on the TensorEngine, reciprocal on VectorE, broadcast across
    partitions via a rank-1 matmul.
  - b_out is folded into the Exp activation bias (per-partition), b_ih+b_hh
    into the Tanh bias, and b_emb is folded on the host into the recurrent
    bias of steps t>=1  (b_h1 = b_ih + b_hh + W_ih @ b_emb), so e never
    carries its bias on device.
  - Matmul operands in MM_DT (bfloat16 or float32r); all softmax arithmetic
    (gumbel add, exp, sums, normalization) in fp32.
"""

import os
import sys

import numpy as np

for _p in ("/root/.axon_site/_ro/trn_rl_repo", "/opt/trn_rl_repo"):
    if _p not in sys.path and os.path.isdir(_p):
        sys.path.append(_p)

import concourse.bass as bass
import concourse.mybir as mybir
import concourse.tile as tile
from concourse.alu_op_type import AluOpType
from concourse.bass_utils import run_bass_kernel_spmd

VOCAB = 1024
HID = 1024
EMB = 256
NOS = 12
N = 4096
NCORES = 8
NS = N // NCORES          # 512 rows per core
P = 128                   # partitions
KH = HID // P             # 8 hid tiles
KV = VOCAB // P           # 8 vocab tiles
KE = EMB // P             # 2 emb tiles
FB = NS                   # batch free dim per core (512)

MM_DT = mybir.dt.bfloat16          # matmul operand dtype
MM_NP = mybir.dt.np(MM_DT)
F32 = mybir.dt.float32
ACT = mybir.ActivationFunctionType


# ---------------------------------------------------------------------------
# Workaround: this walrus build supports only ONE sem wait per instruction
# ("Too many sync wait commands"), while Tile emits multi-wait instructions
# routinely.  Post-pass: move all but the last wait of every instruction onto
# fresh same-engine NoOps inserted immediately before it (same-engine program
# order makes this equivalent).
# ---------------------------------------------------------------------------
import bass_rust as _bass_rust


def split_multi_waits(nc):
    ctr = 0
    for f in nc.m.functions:
        for bb in f.blocks:
            new = []
            changed = False
            for inst in list(bb.instructions):
                si = inst.sync_info
                waits = list(si.on_wait) if si is not None else []
                if len(waits) > 1:
                    changed = True
                    for w in waits[:-1]:
                        nop = _bass_rust.InstNoOp(
                            name=f"I-wsplit-{ctr}", engine=inst.engine
                        )
                        ctr += 1
                        nop.sync_info = mybir.SyncInfo(on_wait=[w], on_update=[])
                        new.append(nop)
                    inst.sync_info = mybir.SyncInfo(
                        on_wait=[waits[-1]], on_update=list(si.on_update)
                    )
                new.append(inst)
            if changed:
                bb.instructions = new
    return ctr


# ---------------------------------------------------------------------------
# Device program (identical on every core; SPMD over the batch axis)
# ---------------------------------------------------------------------------
def emit_body(tc, io):
    """io: dict name -> bass.AP for the dram tensors."""
    nc = tc.nc
    tT, gT = io["tT"], io["gT"]
    whhT, woutT, wihT, wembT = io["whhT"], io["woutT"], io["wihT"], io["wembT"]
    bh, bo, sos = io["bh"], io["bo"], io["sos"]
    xout = io["xout"]

    import contextlib

    with contextlib.ExitStack() as ctx:
        singles = ctx.enter_context(tc.tile_pool(name="singles", bufs=1))
        h_pool = ctx.enter_context(tc.tile_pool(name="h", bufs=2))
        e_pool = ctx.enter_context(tc.tile_pool(name="e", bufs=2))
        u_pool = ctx.enter_context(tc.tile_pool(name="u", bufs=KV + 8))
        g_pool = ctx.enter_context(tc.tile_pool(name="g", bufs=16))
        x_pool = ctx.enter_context(tc.tile_pool(name="x", bufs=KV + 2))
        bc_pool = ctx.enter_context(tc.tile_pool(name="bc", bufs=2))
        rs_pool = ctx.enter_context(tc.tile_pool(name="rs", bufs=2))
        ps_h = ctx.enter_context(tc.tile_pool(name="ps_h", bufs=2, space="PSUM"))
        ps_z = ctx.enter_context(tc.tile_pool(name="ps_z", bufs=3, space="PSUM"))
        ps_s = ctx.enter_context(tc.tile_pool(name="ps_s", bufs=1, space="PSUM"))
        ps_b = ctx.enter_context(tc.tile_pool(name="ps_b", bufs=1, space="PSUM"))
        ps_e = ctx.enter_context(tc.tile_pool(name="ps_e", bufs=1, space="PSUM"))

        # ---- weights / constants into SBUF (persistent) ----
        def load_tiles(src, n_tiles, width, dt, tag):
            ts = []
            for k in range(n_tiles):
                t = singles.tile([P, width], dt, tag=f"{tag}{k}")
                nc.sync.dma_start(out=t, in_=src[k * P:(k + 1) * P, :])
                ts.append(t)
            return ts

        whh_sb = load_tiles(whhT, KH, HID, MM_DT, "whh")
        wout_sb = load_tiles(woutT, KH, VOCAB, MM_DT, "wout")
        wih_sb = load_tiles(wihT, KE, HID, MM_DT, "wih")
        wemb_sb = load_tiles(wembT, KV, EMB, MM_DT, "wemb")
        bh_sb = load_tiles(bh, KH, 2, F32, "bh")
        bo_sb = load_tiles(bo, KV, 1, F32, "bo")
        sos_sb = load_tiles(sos, KE, 1, F32, "sos")

        ones_col = singles.tile([P, 1], MM_DT, tag="ones_col")
        nc.vector.memset(ones_col, 1.0)
        ones_row_f = singles.tile([1, P], F32, tag="ones_row_f")
        nc.vector.memset(ones_row_f, 1.0)
        ones_row = singles.tile([1, P], mybir.dt.float32r, tag="ones_row")
        with nc.allow_low_precision(reason="bit-copy of exact 1.0s to f32r"):
            nc.vector.tensor_copy(ones_row, ones_row_f)
        ones_blk = singles.tile([P, FB], F32, tag="ones_blk")
        nc.vector.memset(ones_blk, 1.0)

        # ---- initial state ----
        # h0 comes transposed from the host; e0 = sos broadcast along batch.
        h_prev = []
        for k in range(KH):
            t = h_pool.tile([P, FB], MM_DT, tag=f"h{k}")
            nc.sync.dma_start(out=t, in_=tT[k * P:(k + 1) * P, :])
            h_prev.append(t)
        e_prev = []
        for k in range(KE):
            t = e_pool.tile([P, FB], MM_DT, tag=f"e{k}")
            nc.scalar.activation(t, ones_blk, ACT.Copy, scale=sos_sb[k][:, 0:1])
            e_prev.append(t)

        # ---- the 12-step scan ----
        for t_step in range(NOS):
            bias_col = 0 if t_step == 0 else 1

            # h = tanh(W_hh h_prev + W_ih e_prev + b)
            h_new = []
            for m in range(KH):
                acc = ps_h.tile([P, FB], F32, tag="ps_h")
                for k in range(KH):
                    nc.tensor.matmul(
                        acc, lhsT=whh_sb[k][:, m * P:(m + 1) * P], rhs=h_prev[k],
                        start=(k == 0), stop=False,
                    )
                for k in range(KE):
                    nc.tensor.matmul(
                        acc, lhsT=wih_sb[k][:, m * P:(m + 1) * P], rhs=e_prev[k],
                        start=False, stop=(k == KE - 1),
                    )
                ht = h_pool.tile([P, FB], MM_DT, tag=f"h{m}")
                nc.scalar.activation(
                    ht, acc, ACT.Tanh, bias=bh_sb[m][:, bias_col:bias_col + 1]
                )
                h_new.append(ht)

            # z = W_out h + b_out + g ;  u = exp(z)
            u = []
            for m in range(KV):
                acc = ps_z.tile([P, FB], F32, tag="ps_z")
                for k in range(KH):
                    nc.tensor.matmul(
                        acc, lhsT=wout_sb[k][:, m * P:(m + 1) * P], rhs=h_new[k],
                        start=(k == 0), stop=(k == KH - 1),
                    )
                gt = g_pool.tile([P, FB], F32, tag="g")
                nc.sync.dma_start(out=gt, in_=gT[t_step, m * P:(m + 1) * P, :])
                nc.vector.tensor_tensor(acc, acc, gt, op=AluOpType.add)
                ut = u_pool.tile([P, FB], MM_DT, tag="u")
                nc.scalar.activation(ut, acc, ACT.Exp, bias=bo_sb[m][:, 0:1])
                u.append(ut)

            # s = column sums of u ; rs = 1/s ; bc = broadcast of rs
            s_ps = ps_s.tile([1, FB], F32, tag="ps_s")
            for k in range(KV):
                nc.tensor.matmul(
                    s_ps, lhsT=ones_col, rhs=u[k],
                    start=(k == 0), stop=(k == KV - 1),
                )
            rs = rs_pool.tile([1, FB], mybir.dt.float32r, tag="rs")
            b_ps = ps_b.tile([P, FB], F32, tag="ps_b")
            with nc.allow_low_precision(reason="f32r rank-1 broadcast of 1/s"):
                nc.vector.reciprocal(rs, s_ps)
                nc.tensor.matmul(b_ps, lhsT=ones_row, rhs=rs, start=True, stop=True)
            bc = bc_pool.tile([P, FB], F32, tag="bc")
            nc.scalar.activation(bc, b_ps, ACT.Copy)

            # x = u * (1/s broadcast)  -> DRAM  (fp32)
            for m in range(KV):
                xt = x_pool.tile([P, FB], F32, tag="x")
                nc.vector.scalar_tensor_tensor(
                    out=xt, in0=u[m], scalar=1.0, in1=b_ps,
                    op0=AluOpType.mult, op1=AluOpType.mult,
                )
                nc.sync.dma_start(
                    out=xout[t_step, m * P:(m + 1) * P, :], in_=xt
                )

            # e = (W_emb u) * bc   (b_emb folded into bh col 1)
            e_new = []
            for m in range(KE):
                acc = ps_e.tile([P, FB], F32, tag="ps_e")
                for k in range(KV):
                    nc.tensor.matmul(
                        acc, lhsT=wemb_sb[k][:, m * P:(m + 1) * P], rhs=u[k],
                        start=(k == 0), stop=(k == KV - 1),
                    )
                et = e_pool.tile([P, FB], MM_DT, tag=f"e{m}")
                nc.vector.scalar_tensor_tensor(
                    out=et, in0=acc, scalar=1.0, in1=bc,
                    op0=AluOpType.mult, op1=AluOpType.mult,
                )
                e_new.append(et)

            h_prev, e_prev = h_new, e_new


# ---------------------------------------------------------------------------
# Graph construction
# ---------------------------------------------------------------------------
def build_nc(reps=1):
    nc = bass.Bass("TRN2", target_bir_lowering=False, debug=False,
                   num_devices=NCORES)
    io = {
        "tT": nc.dram_tensor("tT", [HID, NS], MM_DT, kind="ExternalInput").ap(),
        "gT": nc.dram_tensor("gT", [NOS, VOCAB, NS], F32, kind="ExternalInput").ap(),
        "whhT": nc.dram_tensor("whhT", [HID, HID], MM_DT, kind="ExternalInput").ap(),
        "woutT": nc.dram_tensor("woutT", [HID, VOCAB], MM_DT, kind="ExternalInput").ap(),
        "wihT": nc.dram_tensor("wihT", [EMB, HID], MM_DT, kind="ExternalInput").ap(),
        "wembT": nc.dram_tensor("wembT", [VOCAB, EMB], MM_DT, kind="ExternalInput").ap(),
        "bh": nc.dram_tensor("bh", [HID, 2], F32, kind="ExternalInput").ap(),
        "bo": nc.dram_tensor("bo", [VOCAB, 1], F32, kind="ExternalInput").ap(),
        "sos": nc.dram_tensor("sos", [EMB, 1], F32, kind="ExternalInput").ap(),
        "xout": nc.dram_tensor("xout", [NOS, VOCAB, NS], F32, kind="ExternalOutput").ap(),
    }
    with tile.TileContext(nc) as tc:
        for _ in range(reps):
            emit_body(tc, io)
    n = split_multi_waits(nc)
    print(f"split_multi_waits: {n} nops inserted")
    return nc


# ---------------------------------------------------------------------------
# Host side: preprocess -> SPMD run -> gather
# ---------------------------------------------------------------------------
def make_in_maps(target, gumbels, sos, W_ih, b_ih, W_hh, b_hh, W_out, b_out,
                 W_emb, b_emb):
    f32 = np.float32
    target = np.asarray(target, f32).reshape(N, HID)
    gumbels = np.asarray(gumbels, f32)
    W_ih = np.asarray(W_ih, f32)
    W_hh = np.asarray(W_hh, f32)
    W_out = np.asarray(W_out, f32)
    W_emb = np.asarray(W_emb, f32)
    b_ih = np.asarray(b_ih, f32)
    b_hh = np.asarray(b_hh, f32)
    b_out = np.asarray(b_out, f32)
    b_emb = np.asarray(b_emb, f32)
    sos = np.asarray(sos, f32)

    tT = np.ascontiguousarray(target.T).astype(MM_NP)          # [HID, N]
    gT = np.ascontiguousarray(gumbels.transpose(0, 2, 1))      # [NOS, V, N]

    bh0 = b_ih + b_hh
    bh1 = bh0 + W_ih @ b_emb
    shared = {
        "whhT": np.ascontiguousarray(W_hh.T).astype(MM_NP),
        "woutT": np.ascontiguousarray(W_out.T).astype(MM_NP),
        "wihT": np.ascontiguousarray(W_ih.T).astype(MM_NP),
        "wembT": np.ascontiguousarray(W_emb.T).astype(MM_NP),
        "bh": np.ascontiguousarray(np.stack([bh0, bh1], axis=1)),
        "bo": np.ascontiguousarray(b_out[:, None]),
        "sos": np.ascontiguousarray(sos[:, None]),
    }
    in_maps = []
    for c in range(NCORES):
        sl = slice(c * NS, (c + 1) * NS)
        m = dict(shared)
        m["tT"] = np.ascontiguousarray(tT[:, sl])
        m["gT"] = np.ascontiguousarray(gT[:, :, sl])
        in_maps.append(m)
    return in_maps


def gather_out(results):
    full = np.concatenate([r["xout"] for r in results], axis=2)  # [NOS, V, N]
    return np.ascontiguousarray(full.transpose(2, 0, 1))         # [N, NOS, V]


_NC_CACHE = {}


def get_nc():
    if "nc" not in _NC_CACHE:
        _NC_CACHE["nc"] = build_nc()
    return _NC_CACHE["nc"]


def kernel(**inputs) -> np.ndarray:
    nc = get_nc()
    in_maps = make_in_maps(**inputs)
    res = run_bass_kernel_spmd(nc, in_maps, list(range(NCORES)))
    return gather_out(res.results)

